# revision 37
# baseline (speedup 1.0000x reference)
"""Trainium2 Bass kernel for nn_DKOKernel (dense pairwise MLP + PSD head).

Math (per batch b, data-parallel: core b <- batch b):
  hx[f,i] = wx x_i;  hy[f,j] = wy y_j       (BN1 folded into wx/wy/c1)
  h1 = relu(hx_i + hy_j)                    (512)
  h2 = relu(W2' h1 + c2)                    (256)
  h3 = relu(W3' h2 + c3)                    (128)
  out[i,j] = h3_ij . u_i + d_i
  u_i = G q_i + g0,  G = W4^T W4,  g0 = ny W4^T b4,  q_i = sum_j h3_ij
  d_i = wc.q_i + ny|b4|^2  (applied on HOST from the shipped q)

Engine budget per 512-pair chunk (HW-measured costs):
  PE  : L2 8 + L3 2 + pf4 1 bf16 matmuls (ap=512) + ~4 LDWEIGHTS
  DVE : 10 narrow L1 fused add+relu (244) + q 4-chunk reduce + pf4 evac
  ACT : 5 narrow L1 (400) + h2 evacs (662) + h3 evac (662)
  Pool: 1 narrow L1 (GpSimd cannot touch PSUM; narrow TS ~2us there)
pf4 trick: the 4 head matmuls of a superchunk land in ONE PSUM bank at
partition offsets 0/32/64/96 (tile_position), so a single 512-col DVE
copy evacuates 4 chunks of output; one strided DMA writes all rows.
"""

import numpy as np
from contextlib import ExitStack

import concourse.bacc as bacc
import concourse.tile as tile
from concourse import mybir
from concourse.bass_utils import run_bass_kernel_spmd

F32 = mybir.dt.float32
BF16 = mybir.dt.bfloat16
AF = mybir.ActivationFunctionType
ALU = mybir.AluOpType
AX = mybir.AxisListType

EPS = 1e-5
B = 8
N = 128          # nx == ny
F = 128          # input feature dim
D1, D2, D3, D4 = 512, 256, 128, 64
C = 4            # i-rows per chunk -> 512 pairs per chunk
NCH = N // C     # 32 chunks
SUB = 4          # chunks per superchunk
NSUP = NCH // SUB


def build_module():
    nc = bacc.Bacc()

    xT = nc.declare_dram_parameter("xT", [F, N], BF16, isOutput=False)
    yT = nc.declare_dram_parameter("yT", [F, N], BF16, isOutput=False)
    wxT = nc.declare_dram_parameter("wxT", [F, D1], BF16, isOutput=False)
    wyT = nc.declare_dram_parameter("wyT", [F, D1], BF16, isOutput=False)
    w2T = nc.declare_dram_parameter("w2T", [128, 4 * D2], BF16, isOutput=False)
    w3T = nc.declare_dram_parameter("w3T", [128, 2 * D3], BF16, isOutput=False)
    gmat = nc.declare_dram_parameter("gmat", [128, 128], BF16, isOutput=False)
    g0d = nc.declare_dram_parameter("g0", [128], F32, isOutput=False)
    c1d = nc.declare_dram_parameter("c1", [128, 4], F32, isOutput=False)
    c2d = nc.declare_dram_parameter("c2", [128, 2], F32, isOutput=False)
    c3d = nc.declare_dram_parameter("c3", [128], F32, isOutput=False)
    out_d = nc.declare_dram_parameter("out", [N, N], F32, isOutput=True)
    q_d = nc.declare_dram_parameter("qout", [128, N], F32, isOutput=True)

    with tile.TileContext(nc) as tc:
        with ExitStack() as ctx:
            singles = ctx.enter_context(tc.tile_pool(name="singles", bufs=1))

            xT_s = singles.tile([F, N], BF16)
            yT_s = singles.tile([F, N], BF16)
            wxT_s = singles.tile([F, D1], BF16)
            wyT_s = singles.tile([F, D1], BF16)
            w2_s = singles.tile([128, 4, D2], BF16)
            w3_s = singles.tile([128, 2, D3], BF16)
            g_s = singles.tile([128, 128], BF16)
            g0_s = singles.tile([128, 1], F32)
            c1_s = singles.tile([128, 4], F32)
            c2_s = singles.tile([128, 2], F32)
            c3_s = singles.tile([128, 1], F32)
            hx_s = singles.tile([128, 4, N], F32)
            hy_s = singles.tile([128, 4, N], BF16)
            q_all = singles.tile([128, N], F32)
            u_all = singles.tile([128, N], BF16)
            out4_s = singles.tile([4, NCH, C * N], F32)

            nc.sync.dma_start(out=wxT_s, in_=wxT[:, :])
            nc.sync.dma_start(out=xT_s, in_=xT[:, :])
            nc.sync.dma_start(out=wyT_s, in_=wyT[:, :])
            nc.sync.dma_start(out=yT_s, in_=yT[:, :])
            nc.sync.dma_start(out=c1_s, in_=c1d[:, :])
            nc.sync.dma_start(
                out=w2_s.rearrange("p a b -> p (a b)"), in_=w2T[:, :])
            nc.sync.dma_start(
                out=w3_s.rearrange("p a b -> p (a b)"), in_=w3T[:, :])
            nc.sync.dma_start(out=c2_s, in_=c2d[:, :])
            nc.sync.dma_start(out=c3_s[:, 0], in_=c3d[:])
            nc.sync.dma_start(out=g_s, in_=gmat[:, :])
            nc.sync.dma_start(out=g0_s[:, 0], in_=g0d[:])

            # ---- setup: hx (fp32), hy (bf16, +c1) ----
            with tc.tile_pool(name="psum_setup", bufs=2, space="PSUM") as pp:
                for fc in range(4):
                    ph = pp.tile([128, N], F32, tag="ph")
                    nc.tensor.matmul(
                        ph, lhsT=wxT_s[:, fc * 128:(fc + 1) * 128],
                        rhs=xT_s, start=True, stop=True)
                    nc.scalar.activation(hx_s[:, fc, :], ph, AF.Copy)
                    py_ = pp.tile([128, N], F32, tag="ph")
                    nc.tensor.matmul(
                        py_, lhsT=wyT_s[:, fc * 128:(fc + 1) * 128],
                        rhs=yT_s, start=True, stop=True)
                    nc.scalar.activation(hy_s[:, fc, :], py_, AF.Identity,
                                         bias=c1_s[:, fc:fc + 1])

            work = ctx.enter_context(tc.tile_pool(name="work", bufs=3))
            h1p = ctx.enter_context(tc.tile_pool(name="h1p", bufs=4 * SUB))
            h2p = ctx.enter_context(tc.tile_pool(name="h2p", bufs=SUB + 2))
            h3p = ctx.enter_context(tc.tile_pool(name="h3p", bufs=5))
            psA = ctx.enter_context(tc.tile_pool(name="psA", bufs=4,
                                                 space="PSUM"))
            psB = ctx.enter_context(tc.tile_pool(name="psB", bufs=2,
                                                 space="PSUM"))
            psC = ctx.enter_context(tc.tile_pool(name="psC", bufs=2,
                                                 space="PSUM"))

            def emit_L1(ts):
                """h1 for a superchunk, fc-major. DVE 10 / ACT 5 / Pool 1
                narrow fused add+relu ops per chunk."""
                h1s = [[h1p.tile([128, C * N], BF16, name="h1", tag="h1")
                        for _ in range(4)] for _ in ts]
                for fc in range(4):
                    for s, t in enumerate(ts):
                        for ii in range(C):
                            xc = hx_s[:, fc, C * t + ii:C * t + ii + 1]
                            dst = h1s[s][fc][:, ii * N:(ii + 1) * N]
                            if fc == 3 and ii == 3:
                                nc.gpsimd.tensor_scalar(
                                    out=dst, in0=hy_s[:, fc, :], scalar1=xc,
                                    scalar2=0.0, op0=ALU.add, op1=ALU.max)
                            elif (fc == 2 and ii >= 2) or \
                                 (fc == 3 and ii in (0, 1, 2)):
                                nc.scalar.activation(
                                    dst, hy_s[:, fc, :], AF.Relu, bias=xc)
                            else:
                                nc.vector.tensor_scalar(
                                    out=dst, in0=hy_s[:, fc, :], scalar1=xc,
                                    scalar2=0.0, op0=ALU.add, op1=ALU.max)
                return h1s

            def emit_L2(h1s):
                """L2: mc-outer, kc-outer, chunk-inner (4-chunk stationary
                reuse, 8 LDW per superchunk). Per-(chunk,mc) ACT evac."""
                h2s = [h2p.tile([128, 2, C * N], BF16, name="h2", tag="h2")
                       for _ in h1s]
                for mc in range(2):
                    p2s = [psA.tile([128, C * N], F32, name="p2", tag="p2")
                           for _ in h1s]
                    for kc in range(4):
                        for s in range(len(h1s)):
                            mm = nc.tensor.matmul(
                                p2s[s],
                                lhsT=w2_s[:, kc, mc * 128:(mc + 1) * 128],
                                rhs=h1s[s][kc],
                                start=(kc == 0), stop=(kc == 3))
                            if s > 0:
                                mm.ins.ldweights = False
                    for s in range(len(h1s)):
                        nc.scalar.activation(
                            h2s[s][:, mc, :], p2s[s], AF.Relu,
                            bias=c2_s[:, mc:mc + 1])
                return h2s

            def emit_L3(sp, h2s):
                """L3 (w3 stationary reused across chunk pairs) + per-chunk
                ACT h3 evac into a superchunk-wide tile + one DVE q-reduce."""
                h3q_t = h3p.tile([128, SUB, C * N], BF16, name="h3",
                                 tag="h3")
                for pair in range(len(h2s) // 2):
                    p3s = [psB.tile([128, C * N], F32, name="p3", tag="p3")
                           for _ in range(2)]
                    for kc in range(2):
                        for s in range(2):
                            mm = nc.tensor.matmul(
                                p3s[s], lhsT=w3_s[:, kc, :],
                                rhs=h2s[2 * pair + s][:, kc, :],
                                start=(kc == 0), stop=(kc == 1))
                            if s > 0:
                                mm.ins.ldweights = False
                    for s in range(2):
                        nc.scalar.activation(
                            h3q_t[:, 2 * pair + s, :], p3s[s], AF.Relu,
                            bias=c3_s)
                i0 = C * SUB * sp
                nc.vector.tensor_reduce(
                    out=q_all[:, i0:i0 + C * SUB],
                    in_=h3q_t.rearrange("p s (a b) -> p (s a) b", a=C),
                    axis=AX.X, op=ALU.add)
                return h3q_t

            def emit_batch_su(sp):
                """u = G q + g0 for the 16 i's of superchunk sp."""
                i0 = C * SUB * sp
                nb = C * SUB
                qb = work.tile([128, nb], BF16, tag="qb")
                nc.vector.tensor_copy(out=qb, in_=q_all[:, i0:i0 + nb])
                pu_big = psA.tile([128, C * N], F32, name="p2", tag="p2")
                p_u = pu_big[:, 0:nb]
                nc.tensor.matmul(p_u, lhsT=g_s, rhs=qb,
                                 start=True, stop=True)
                nc.vector.tensor_scalar(
                    out=u_all[:, i0:i0 + nb], in0=p_u, scalar1=g0_s,
                    scalar2=0.0, op0=ALU.add, op1=ALU.add)

            def emit_head(sp, h3q_t):
                """pf4 = u_chunk^T . h3 per chunk; [4,512] evac staged to
                out4_s (diagonal row ii lives at free offset ii*N)."""
                for s in range(SUB):
                    t = SUB * sp + s
                    ph = psC.tile([C, C * N], F32, name="ph4", tag="ph4")
                    nc.tensor.matmul(
                        ph, lhsT=u_all[:, C * t:C * t + C],
                        rhs=h3q_t[:, s, :], start=True, stop=True)
                    if t % 2 == 0:
                        nc.vector.tensor_copy(out=out4_s[:, t, :], in_=ph)
                    else:
                        nc.scalar.activation(out4_s[:, t, :], ph, AF.Copy)

            # -------- main pipeline (batch lags 2, head lags 3) --------
            h3q = {}
            for sp in range(NSUP):
                if sp >= 2:
                    emit_batch_su(sp - 2)
                h1s = emit_L1([SUB * sp + s for s in range(SUB)])
                h2s = emit_L2(h1s)
                h3q[sp] = emit_L3(sp, h2s)
                if sp >= 3:
                    emit_head(sp - 3, h3q.pop(sp - 3))
            emit_batch_su(NSUP - 2)
            emit_head(NSUP - 3, h3q.pop(NSUP - 3))
            emit_batch_su(NSUP - 1)
            emit_head(NSUP - 2, h3q.pop(NSUP - 2))
            emit_head(NSUP - 1, h3q.pop(NSUP - 1))

            nc.sync.dma_start(out=q_d[:, :], in_=q_all)
            # out4_s[ii, t, ii*N + j] = out[4t+ii, j]
            ov = out_d.rearrange("(t a) j -> a t j", a=C)
            o4 = out4_s.rearrange("p t (a j) -> p a t j", j=N)
            for r in range(C):
                nc.sync.dma_start(out=ov[r, :, :].unsqueeze(0),
                                  in_=o4[r:r + 1, r, :, :])
    nc.finalize()
    return nc


_NC_CACHE = None


def _get_nc():
    global _NC_CACHE
    if _NC_CACHE is None:
        _NC_CACHE = build_module()
    return _NC_CACHE


def host_prep(inputs):
    """Fold BN affines; pre-transpose to device layouts (bf16 matmul
    operands). Returns (per-core input maps, wc, c0) — the host applies
    out += d_i with d = wc.q + c0 after the kernel."""
    f32 = np.float32
    x = np.asarray(inputs["x"], f32)
    y = np.asarray(inputs["y"], f32)
    w1, b1 = np.asarray(inputs["w1"], f32), np.asarray(inputs["b1"], f32)
    w2, b2 = np.asarray(inputs["w2"], f32), np.asarray(inputs["b2"], f32)
    w3, b3 = np.asarray(inputs["w3"], f32), np.asarray(inputs["b3"], f32)
    w4, b4 = np.asarray(inputs["w4"], f32), np.asarray(inputs["b4"], f32)

    k1 = inputs["g1"] / np.sqrt(inputs["v1"] + EPS)
    c1 = k1 * (b1 - inputs["m1"]) + inputs["be1"]
    k2 = inputs["g2"] / np.sqrt(inputs["v2"] + EPS)
    c2 = k2 * (b2 - inputs["m2"]) + inputs["be2"]
    k3 = inputs["g3"] / np.sqrt(inputs["v3"] + EPS)
    c3 = k3 * (b3 - inputs["m3"]) + inputs["be3"]

    wx = w1[:, :F] * k1[:, None]          # (512, 128)
    wy = w1[:, F:] * k1[:, None]
    w2f = w2 * k2[:, None]                # (256, 512)
    w3f = w3 * k3[:, None]                # (128, 256)

    import ml_dtypes

    def to_bf16(a):
        return np.ascontiguousarray(np.asarray(a, np.float32)).astype(
            ml_dtypes.bfloat16)

    shared = {
        "wxT": to_bf16(wx.T.copy()),                     # (128, 512)
        "wyT": to_bf16(wy.T.copy()),
        "w2T": to_bf16(w2f.T.reshape(4, 128, D2).transpose(1, 0, 2)
                       .reshape(128, 4 * D2).copy()),
        "w3T": to_bf16(w3f.T.reshape(2, 128, D3).transpose(1, 0, 2)
                       .reshape(128, 2 * D3).copy()),
        "gmat": to_bf16((w4.T @ w4).copy()),             # (128, 128)
        "g0": np.ascontiguousarray(N * (w4.T @ b4), f32),
        "c1": np.ascontiguousarray(c1.reshape(4, 128).T, f32),
        "c2": np.ascontiguousarray(c2.reshape(2, 128).T, f32),
        "c3": np.ascontiguousarray(c3, f32),
    }
    in_maps = []
    for b in range(B):
        m = dict(shared)
        m["xT"] = to_bf16(x[b].T.copy())
        m["yT"] = to_bf16(y[b].T.copy())
        in_maps.append(m)
    wc = (w4.T @ b4).astype(f32)                          # (128,)
    c0 = np.float32(N * float(b4 @ b4))
    return in_maps, wc, c0


def kernel(**inputs):
    nc = _get_nc()
    in_maps, wc, c0 = host_prep(inputs)
    res = run_bass_kernel_spmd(nc, in_maps, list(range(B)))
    outs = []
    for b in range(B):
        pf = res.results[b]["out"]                        # (128, 128)
        q = res.results[b]["qout"]                        # (128 f, 128 i)
        d = wc @ q + c0                                   # (128 i,)
        outs.append(pf + d[None, :].T)
    return np.stack(outs, axis=0).astype(np.float32)


# revision 38
# speedup vs baseline: 1.2986x; 1.2986x over previous
"""Trainium2 Bass kernel for nn_DKOKernel (dense pairwise MLP + PSD head).

Math (per batch b, data-parallel: core b <- batch b):
  hx[f,i] = wx x_i;  hy[f,j] = wy y_j       (BN1 folded into wx/wy/c1)
  h1 = relu(hx_i + hy_j)                    (512)
  h2 = relu(W2' h1 + c2)                    (256)
  h3 = relu(W3' h2 + c3)                    (128)
  out[i,j] = h3_ij . u_i + d_i
  u_i = G q_i + g0,  G = W4^T W4,  g0 = ny W4^T b4,  q_i = sum_j h3_ij
  d_i = wc.q_i + ny|b4|^2  (applied on HOST from the shipped q)

Engine budget per 512-pair chunk (HW-measured costs):
  PE  : L2 8 + L3 2 + pf4 1 bf16 matmuls (ap=512) + ~4 LDWEIGHTS
  DVE : 10 narrow L1 fused add+relu (244) + q 4-chunk reduce + pf4 evac
  ACT : 5 narrow L1 (400) + h2 evacs (662) + h3 evac (662)
  Pool: 1 narrow L1 (GpSimd cannot touch PSUM; narrow TS ~2us there)
pf4 trick: the 4 head matmuls of a superchunk land in ONE PSUM bank at
partition offsets 0/32/64/96 (tile_position), so a single 512-col DVE
copy evacuates 4 chunks of output; one strided DMA writes all rows.
"""

import numpy as np
from contextlib import ExitStack

import concourse.bacc as bacc
import concourse.tile as tile
from concourse import mybir
from concourse.bass_utils import run_bass_kernel_spmd

F32 = mybir.dt.float32
BF16 = mybir.dt.bfloat16
AF = mybir.ActivationFunctionType
ALU = mybir.AluOpType
AX = mybir.AxisListType

EPS = 1e-5
B = 8
N = 128          # nx == ny
F = 128          # input feature dim
D1, D2, D3, D4 = 512, 256, 128, 64
C = 4            # i-rows per chunk -> 512 pairs per chunk
NCH = N // C     # 32 chunks
SUB = 4          # chunks per superchunk
NSUP = NCH // SUB


def build_module():
    nc = bacc.Bacc()

    xT = nc.declare_dram_parameter("xT", [F, N], BF16, isOutput=False)
    yT = nc.declare_dram_parameter("yT", [F, N], BF16, isOutput=False)
    wxT = nc.declare_dram_parameter("wxT", [F, D1], BF16, isOutput=False)
    wyT = nc.declare_dram_parameter("wyT", [F, D1], BF16, isOutput=False)
    w2T = nc.declare_dram_parameter("w2T", [128, 4 * D2], BF16, isOutput=False)
    w3T = nc.declare_dram_parameter("w3T", [128, 2 * D3], BF16, isOutput=False)
    gmat = nc.declare_dram_parameter("gmat", [128, 128], BF16, isOutput=False)
    g0d = nc.declare_dram_parameter("g0", [128], F32, isOutput=False)
    c1d = nc.declare_dram_parameter("c1", [128, 4], F32, isOutput=False)
    c2d = nc.declare_dram_parameter("c2", [128, 2], F32, isOutput=False)
    c3d = nc.declare_dram_parameter("c3", [128], F32, isOutput=False)
    out_d = nc.declare_dram_parameter("out", [N, N], F32, isOutput=True)
    q_d = nc.declare_dram_parameter("qout", [128, N], F32, isOutput=True)

    with tile.TileContext(nc) as tc:
        with ExitStack() as ctx:
            singles = ctx.enter_context(tc.tile_pool(name="singles", bufs=1))

            xT_s = singles.tile([F, N], BF16)
            yT_s = singles.tile([F, N], BF16)
            wxT_s = singles.tile([F, D1], BF16)
            wyT_s = singles.tile([F, D1], BF16)
            w2_s = singles.tile([128, 4, D2], BF16)
            w3_s = singles.tile([128, 2, D3], BF16)
            g_s = singles.tile([128, 128], BF16)
            g0_s = singles.tile([128, 1], F32)
            c1_s = singles.tile([128, 4], F32)
            c2_s = singles.tile([128, 2], F32)
            c3_s = singles.tile([128, 1], F32)
            hx_s = singles.tile([128, 4, N], F32)
            hy_s = singles.tile([128, 4, N], BF16)
            q_all = singles.tile([128, N], F32)
            u_all = singles.tile([128, N], BF16)
            out4_s = singles.tile([4, NCH, C * N], F32)

            nc.sync.dma_start(out=wxT_s, in_=wxT[:, :])
            nc.sync.dma_start(out=xT_s, in_=xT[:, :])
            nc.sync.dma_start(out=wyT_s, in_=wyT[:, :])
            nc.sync.dma_start(out=yT_s, in_=yT[:, :])
            nc.sync.dma_start(out=c1_s, in_=c1d[:, :])
            nc.sync.dma_start(
                out=w2_s.rearrange("p a b -> p (a b)"), in_=w2T[:, :])
            nc.sync.dma_start(
                out=w3_s.rearrange("p a b -> p (a b)"), in_=w3T[:, :])
            nc.sync.dma_start(out=c2_s, in_=c2d[:, :])
            nc.sync.dma_start(out=c3_s[:, 0], in_=c3d[:])
            nc.sync.dma_start(out=g_s, in_=gmat[:, :])
            nc.sync.dma_start(out=g0_s[:, 0], in_=g0d[:])

            # ---- setup: hx (fp32), hy (bf16, +c1) ----
            with tc.tile_pool(name="psum_setup", bufs=2, space="PSUM") as pp:
                for fc in range(4):
                    ph = pp.tile([128, N], F32, tag="ph")
                    nc.tensor.matmul(
                        ph, lhsT=wxT_s[:, fc * 128:(fc + 1) * 128],
                        rhs=xT_s, start=True, stop=True)
                    nc.scalar.activation(hx_s[:, fc, :], ph, AF.Copy)
                    py_ = pp.tile([128, N], F32, tag="ph")
                    nc.tensor.matmul(
                        py_, lhsT=wyT_s[:, fc * 128:(fc + 1) * 128],
                        rhs=yT_s, start=True, stop=True)
                    nc.scalar.activation(hy_s[:, fc, :], py_, AF.Identity,
                                         bias=c1_s[:, fc:fc + 1])

            work = ctx.enter_context(tc.tile_pool(name="work", bufs=3))
            h1p = ctx.enter_context(tc.tile_pool(name="h1p", bufs=8 * SUB))
            h2p = ctx.enter_context(tc.tile_pool(name="h2p", bufs=SUB + 2))
            h3p = ctx.enter_context(tc.tile_pool(name="h3p", bufs=5))
            psA = ctx.enter_context(tc.tile_pool(name="psA", bufs=4,
                                                 space="PSUM"))
            psB = ctx.enter_context(tc.tile_pool(name="psB", bufs=2,
                                                 space="PSUM"))
            psC = ctx.enter_context(tc.tile_pool(name="psC", bufs=2,
                                                 space="PSUM"))

            def emit_L1(ts):
                """h1 for a superchunk, fc-major. DVE 10 / ACT 5 / Pool 1
                narrow fused add+relu ops per chunk."""
                h1s = [[h1p.tile([128, C * N], BF16, name="h1", tag="h1")
                        for _ in range(4)] for _ in ts]
                for fc in range(4):
                    for s, t in enumerate(ts):
                        for ii in range(C):
                            xc = hx_s[:, fc, C * t + ii:C * t + ii + 1]
                            dst = h1s[s][fc][:, ii * N:(ii + 1) * N]
                            if fc == 3 and ii == 3:
                                nc.gpsimd.tensor_scalar(
                                    out=dst, in0=hy_s[:, fc, :], scalar1=xc,
                                    scalar2=0.0, op0=ALU.add, op1=ALU.max)
                            elif (fc == 2 and ii >= 2) or \
                                 (fc == 3 and ii in (0, 1, 2)):
                                nc.scalar.activation(
                                    dst, hy_s[:, fc, :], AF.Relu, bias=xc)
                            else:
                                nc.vector.tensor_scalar(
                                    out=dst, in0=hy_s[:, fc, :], scalar1=xc,
                                    scalar2=0.0, op0=ALU.add, op1=ALU.max)
                return h1s

            def emit_L2(h1s):
                """L2: mc-outer, kc-outer, chunk-inner (4-chunk stationary
                reuse, 8 LDW per superchunk). Per-(chunk,mc) ACT evac."""
                h2s = [h2p.tile([128, 2, C * N], BF16, name="h2", tag="h2")
                       for _ in h1s]
                for mc in range(2):
                    p2s = [psA.tile([128, C * N], F32, name="p2", tag="p2")
                           for _ in h1s]
                    for kc in range(4):
                        for s in range(len(h1s)):
                            mm = nc.tensor.matmul(
                                p2s[s],
                                lhsT=w2_s[:, kc, mc * 128:(mc + 1) * 128],
                                rhs=h1s[s][kc],
                                start=(kc == 0), stop=(kc == 3))
                            if s > 0:
                                mm.ins.ldweights = False
                    for s in range(len(h1s)):
                        nc.scalar.activation(
                            h2s[s][:, mc, :], p2s[s], AF.Relu,
                            bias=c2_s[:, mc:mc + 1])
                return h2s

            def emit_L3(sp, h2s):
                """L3 (w3 stationary reused across chunk pairs) + per-chunk
                ACT h3 evac into a superchunk-wide tile + one DVE q-reduce."""
                h3q_t = h3p.tile([128, SUB, C * N], BF16, name="h3",
                                 tag="h3")
                for pair in range(len(h2s) // 2):
                    p3s = [psB.tile([128, C * N], F32, name="p3", tag="p3")
                           for _ in range(2)]
                    for kc in range(2):
                        for s in range(2):
                            mm = nc.tensor.matmul(
                                p3s[s], lhsT=w3_s[:, kc, :],
                                rhs=h2s[2 * pair + s][:, kc, :],
                                start=(kc == 0), stop=(kc == 1))
                            if s > 0:
                                mm.ins.ldweights = False
                    for s in range(2):
                        nc.scalar.activation(
                            h3q_t[:, 2 * pair + s, :], p3s[s], AF.Relu,
                            bias=c3_s)
                i0 = C * SUB * sp
                nc.vector.tensor_reduce(
                    out=q_all[:, i0:i0 + C * SUB],
                    in_=h3q_t.rearrange("p s (a b) -> p (s a) b", a=C),
                    axis=AX.X, op=ALU.add)
                return h3q_t

            def emit_batch_su(sp):
                """u = G q + g0 for the 16 i's of superchunk sp."""
                i0 = C * SUB * sp
                nb = C * SUB
                qb = work.tile([128, nb], BF16, tag="qb")
                nc.vector.tensor_copy(out=qb, in_=q_all[:, i0:i0 + nb])
                pu_big = psA.tile([128, C * N], F32, name="p2", tag="p2")
                p_u = pu_big[:, 0:nb]
                nc.tensor.matmul(p_u, lhsT=g_s, rhs=qb,
                                 start=True, stop=True)
                nc.vector.tensor_scalar(
                    out=u_all[:, i0:i0 + nb], in0=p_u, scalar1=g0_s,
                    scalar2=0.0, op0=ALU.add, op1=ALU.add)

            def emit_head(sp, h3q_t):
                """pf4 = u_chunk^T . h3 per chunk; [4,512] evac staged to
                out4_s (diagonal row ii lives at free offset ii*N)."""
                for s in range(SUB):
                    t = SUB * sp + s
                    ph = psC.tile([C, C * N], F32, name="ph4", tag="ph4")
                    nc.tensor.matmul(
                        ph, lhsT=u_all[:, C * t:C * t + C],
                        rhs=h3q_t[:, s, :], start=True, stop=True)
                    if t % 2 == 0:
                        nc.vector.tensor_copy(out=out4_s[:, t, :], in_=ph)
                    else:
                        nc.scalar.activation(out4_s[:, t, :], ph, AF.Copy)

            # -------- main pipeline (batch lags 2, head lags 3) --------
            h3q = {}
            for sp in range(NSUP):
                if sp >= 2:
                    emit_batch_su(sp - 2)
                h1s = emit_L1([SUB * sp + s for s in range(SUB)])
                h2s = emit_L2(h1s)
                h3q[sp] = emit_L3(sp, h2s)
                if sp >= 3:
                    emit_head(sp - 3, h3q.pop(sp - 3))
            emit_batch_su(NSUP - 2)
            emit_head(NSUP - 3, h3q.pop(NSUP - 3))
            emit_batch_su(NSUP - 1)
            emit_head(NSUP - 2, h3q.pop(NSUP - 2))
            emit_head(NSUP - 1, h3q.pop(NSUP - 1))

            nc.sync.dma_start(out=q_d[:, :], in_=q_all)
            # out4_s[ii, t, ii*N + j] = out[4t+ii, j]
            ov = out_d.rearrange("(t a) j -> a t j", a=C)
            o4 = out4_s.rearrange("p t (a j) -> p a t j", j=N)
            for r in range(C):
                nc.sync.dma_start(out=ov[r, :, :].unsqueeze(0),
                                  in_=o4[r:r + 1, r, :, :])
    nc.finalize()
    return nc


_NC_CACHE = None


def _get_nc():
    global _NC_CACHE
    if _NC_CACHE is None:
        _NC_CACHE = build_module()
    return _NC_CACHE


def host_prep(inputs):
    """Fold BN affines; pre-transpose to device layouts (bf16 matmul
    operands). Returns (per-core input maps, wc, c0) — the host applies
    out += d_i with d = wc.q + c0 after the kernel."""
    f32 = np.float32
    x = np.asarray(inputs["x"], f32)
    y = np.asarray(inputs["y"], f32)
    w1, b1 = np.asarray(inputs["w1"], f32), np.asarray(inputs["b1"], f32)
    w2, b2 = np.asarray(inputs["w2"], f32), np.asarray(inputs["b2"], f32)
    w3, b3 = np.asarray(inputs["w3"], f32), np.asarray(inputs["b3"], f32)
    w4, b4 = np.asarray(inputs["w4"], f32), np.asarray(inputs["b4"], f32)

    k1 = inputs["g1"] / np.sqrt(inputs["v1"] + EPS)
    c1 = k1 * (b1 - inputs["m1"]) + inputs["be1"]
    k2 = inputs["g2"] / np.sqrt(inputs["v2"] + EPS)
    c2 = k2 * (b2 - inputs["m2"]) + inputs["be2"]
    k3 = inputs["g3"] / np.sqrt(inputs["v3"] + EPS)
    c3 = k3 * (b3 - inputs["m3"]) + inputs["be3"]

    wx = w1[:, :F] * k1[:, None]          # (512, 128)
    wy = w1[:, F:] * k1[:, None]
    w2f = w2 * k2[:, None]                # (256, 512)
    w3f = w3 * k3[:, None]                # (128, 256)

    import ml_dtypes

    def to_bf16(a):
        return np.ascontiguousarray(np.asarray(a, np.float32)).astype(
            ml_dtypes.bfloat16)

    shared = {
        "wxT": to_bf16(wx.T.copy()),                     # (128, 512)
        "wyT": to_bf16(wy.T.copy()),
        "w2T": to_bf16(w2f.T.reshape(4, 128, D2).transpose(1, 0, 2)
                       .reshape(128, 4 * D2).copy()),
        "w3T": to_bf16(w3f.T.reshape(2, 128, D3).transpose(1, 0, 2)
                       .reshape(128, 2 * D3).copy()),
        "gmat": to_bf16((w4.T @ w4).copy()),             # (128, 128)
        "g0": np.ascontiguousarray(N * (w4.T @ b4), f32),
        "c1": np.ascontiguousarray(c1.reshape(4, 128).T, f32),
        "c2": np.ascontiguousarray(c2.reshape(2, 128).T, f32),
        "c3": np.ascontiguousarray(c3, f32),
    }
    in_maps = []
    for b in range(B):
        m = dict(shared)
        m["xT"] = to_bf16(x[b].T.copy())
        m["yT"] = to_bf16(y[b].T.copy())
        in_maps.append(m)
    wc = (w4.T @ b4).astype(f32)                          # (128,)
    c0 = np.float32(N * float(b4 @ b4))
    return in_maps, wc, c0


def kernel(**inputs):
    nc = _get_nc()
    in_maps, wc, c0 = host_prep(inputs)
    res = run_bass_kernel_spmd(nc, in_maps, list(range(B)))
    outs = []
    for b in range(B):
        pf = res.results[b]["out"]                        # (128, 128)
        q = res.results[b]["qout"]                        # (128 f, 128 i)
        d = wc @ q + c0                                   # (128 i,)
        outs.append(pf + d[None, :].T)
    return np.stack(outs, axis=0).astype(np.float32)
